# revision 14
# baseline (speedup 1.0000x reference)
"""ContextNorm (row-wise layernorm w/ ddof=1 + diag scale + bias) on 8 TRN2 cores.

out = (X - mean(X, axis=1)) / std(X, axis=1, ddof=1) * weights + bias

Sharding: data-parallel over the batch axis (65536 rows -> 8192 rows/core);
weights/bias replicated. Per core: 16 tiles of [128 partitions x 4 rows x 1024]
(16KB contiguous per partition per DMA). Stats via bn_stats/bn_aggr (DVE),
normalize on ACT (in place), *W on DVE (in place), +B on GPSIMD; loads ride
the SP HWDGE ring, stores the ACT HWDGE ring.
"""

import sys

sys.path.insert(0, "/opt/trn_rl_repo")

import numpy as np

import concourse.bass as bass
import concourse.bacc as bacc
import concourse.tile as tile
from concourse import mybir
from concourse.bass_utils import run_bass_kernel_spmd

N_CORES = 8
BATCH = 65536
D = 1024
ROWS = BATCH // N_CORES  # 8192 rows per core
P = 128
R = 4  # rows per partition per tile
N_TILES = ROWS // (P * R)  # 16
F32 = mybir.dt.float32
DDOF_SCALE = float(D) / float(D - 1)  # biased var -> unbiased (ddof=1)

_nc_cache = None


def _broadcast_ap(ap: bass.AP, p: int, r: int) -> bass.AP:
    # Replicate a 1-D [D] DRAM tensor to [p, r, D] (step-0 leading dims).
    return bass.AP(tensor=ap.tensor, offset=ap.offset, ap=[[0, p], [0, r], *ap.ap])


def _build_nc() -> bass.Bass:
    nc = bacc.Bacc("TRN2", target_bir_lowering=False)
    X = nc.dram_tensor("X", [ROWS, D], F32, kind="ExternalInput")
    W = nc.dram_tensor("W", [D], F32, kind="ExternalInput")
    B = nc.dram_tensor("B", [D], F32, kind="ExternalInput")
    O = nc.dram_tensor("out", [ROWS, D], F32, kind="ExternalOutput")

    x_t = X[:, :].rearrange("(n p r) d -> n p r d", p=P, r=R)
    o_t = O[:, :].rearrange("(n p r) d -> n p r d", p=P, r=R)

    with tile.TileContext(nc) as tc:
        with (
            tc.tile_pool(name="consts", bufs=1) as consts,
            tc.tile_pool(name="xio", bufs=5) as xio,
            tc.tile_pool(name="oio", bufs=4) as oio,
            tc.tile_pool(name="stats", bufs=16) as stats,
        ):
            Wb = consts.tile([P, R, D], F32)
            Bb = consts.tile([P, R, D], F32)
            nc.gpsimd.dma_start(out=Wb, in_=_broadcast_ap(W[:], P, R))
            nc.gpsimd.dma_start(out=Bb, in_=_broadcast_ap(B[:], P, R))

            for i in range(N_TILES):
                xt = xio.tile([P, R, D], F32, tag="x")
                nc.sync.dma_start(out=xt, in_=x_t[i])

                st = stats.tile([P, R, 2, 6], F32, tag="bnst")
                for r in range(R):
                    nc.vector.bn_stats(out=st[:, r, 0, :], in_=xt[:, r, 0:512])
                    nc.vector.bn_stats(out=st[:, r, 1, :], in_=xt[:, r, 512:1024])
                mv = stats.tile([P, R, 2], F32, tag="mv")
                for r in range(R):
                    nc.vector.bn_aggr(out=mv[:, r, :], in_=st[:, r, :, :])

                # std = sqrt(var * n/(n-1)) for all rows in one ACT op
                std = stats.tile([P, R], F32, tag="std")
                nc.scalar.activation(
                    out=std,
                    in_=mv[:, :, 1],
                    func=mybir.ActivationFunctionType.Sqrt,
                    scale=DDOF_SCALE,
                )
                rstd = stats.tile([P, R], F32, tag="rstd")
                nc.vector.reciprocal(out=rstd, in_=std)
                # negm = -mean (ACT); nmr[r] = -m*rstd via ACT Copy(scale=rstd)
                negm = stats.tile([P, R], F32, tag="negm")
                nc.scalar.activation(
                    out=negm,
                    in_=mv[:, :, 0],
                    func=mybir.ActivationFunctionType.Copy,
                    scale=-1.0,
                )
                nmr = stats.tile([P, R], F32, tag="nmr")
                for r in range(R):
                    nc.scalar.activation(
                        out=nmr[:, r : r + 1],
                        in_=negm[:, r : r + 1],
                        func=mybir.ActivationFunctionType.Copy,
                        scale=rstd[:, r : r + 1],
                    )

                # x = x * rstd + (-m * rstd)  (normalize on ACT, in place)
                for r in range(R):
                    nc.scalar.activation(
                        out=xt[:, r, :],
                        in_=xt[:, r, :],
                        func=mybir.ActivationFunctionType.Identity,
                        bias=nmr[:, r : r + 1],
                        scale=rstd[:, r : r + 1],
                    )
                # x *= w  (DVE, all rows, in place)
                nc.vector.tensor_mul(out=xt, in0=xt, in1=Wb)
                # out = x + b  (GPSIMD, all rows)
                ot = oio.tile([P, R, D], F32, tag="o")
                nc.gpsimd.tensor_add(out=ot, in0=xt, in1=Bb)
                nc.scalar.dma_start(out=o_t[i], in_=ot)

    nc.compile()
    return nc


def _get_nc() -> bass.Bass:
    global _nc_cache
    if _nc_cache is None:
        _nc_cache = _build_nc()
    return _nc_cache


def run(X, weights, bias, **spmd_kwargs):
    """Run on 8 cores; returns (full_output, BassKernelResults)."""
    X = np.ascontiguousarray(X, dtype=np.float32)
    w = np.ascontiguousarray(weights, dtype=np.float32)
    b = np.ascontiguousarray(bias, dtype=np.float32)
    assert X.shape == (BATCH, D) and w.shape == (D,) and b.shape == (D,)

    nc = _get_nc()
    shards = np.split(X, N_CORES, axis=0)
    in_maps = [{"X": shards[c], "W": w, "B": b} for c in range(N_CORES)]
    res = run_bass_kernel_spmd(nc, in_maps, core_ids=list(range(N_CORES)), **spmd_kwargs)
    out = np.concatenate([res.results[c]["out"] for c in range(N_CORES)], axis=0)
    return out, res


def kernel(X, weights, bias):
    out, _ = run(X, weights, bias)
    return out


# revision 16
# speedup vs baseline: 1.1705x; 1.1705x over previous
"""ContextNorm (row-wise layernorm w/ ddof=1 + diag scale + bias) on 8 TRN2 cores.

out = (X - mean(X, axis=1)) / std(X, axis=1, ddof=1) * weights + bias

Sharding: data-parallel over the batch axis (65536 rows -> 8192 rows/core);
weights/bias replicated. Per core: 16 tiles of [128 partitions x 4 rows x 1024]
(16KB contiguous per partition per DMA). Stats via bn_stats/bn_aggr (DVE),
normalize on ACT (in place), *W on DVE (in place), +B on GPSIMD; loads ride
the SP HWDGE ring, stores the ACT HWDGE ring.
"""

import sys

sys.path.insert(0, "/opt/trn_rl_repo")

import numpy as np

import concourse.bass as bass
import concourse.bacc as bacc
import concourse.tile as tile
from concourse import mybir
from concourse.bass_utils import run_bass_kernel_spmd

N_CORES = 8
BATCH = 65536
D = 1024
ROWS = BATCH // N_CORES  # 8192 rows per core
P = 128
R = 4  # rows per partition per tile
N_TILES = ROWS // (P * R)  # 16
F32 = mybir.dt.float32
DDOF_SCALE = float(D) / float(D - 1)  # biased var -> unbiased (ddof=1)

_nc_cache = None


def _broadcast_ap(ap: bass.AP, p: int, r: int) -> bass.AP:
    # Replicate a 1-D [D] DRAM tensor to [p, r, D] (step-0 leading dims).
    return bass.AP(tensor=ap.tensor, offset=ap.offset, ap=[[0, p], [0, r], *ap.ap])


def _build_nc() -> bass.Bass:
    nc = bacc.Bacc("TRN2", target_bir_lowering=False)
    X = nc.dram_tensor("X", [ROWS, D], F32, kind="ExternalInput")
    W = nc.dram_tensor("W", [D], F32, kind="ExternalInput")
    B = nc.dram_tensor("B", [D], F32, kind="ExternalInput")
    O = nc.dram_tensor("out", [ROWS, D], F32, kind="ExternalOutput")

    x_t = X[:, :].rearrange("(n p r) d -> n p r d", p=P, r=R)
    o_t = O[:, :].rearrange("(n p r) d -> n p r d", p=P, r=R)

    with tile.TileContext(nc) as tc:
        with (
            tc.tile_pool(name="consts", bufs=1) as consts,
            tc.tile_pool(name="xio", bufs=4) as xio,
            tc.tile_pool(name="tmp", bufs=5) as tmp,
            tc.tile_pool(name="stats", bufs=16) as stats,
        ):
            Wb = consts.tile([P, R, D], F32)
            Bb = consts.tile([P, R, D], F32)
            nc.gpsimd.dma_start(out=Wb, in_=_broadcast_ap(W[:], P, R))
            nc.gpsimd.dma_start(out=Bb, in_=_broadcast_ap(B[:], P, R))

            for i in range(N_TILES):
                xt = xio.tile([P, R, D], F32, tag="x")
                nc.sync.dma_start(out=xt, in_=x_t[i])

                st = stats.tile([P, R, 2, 6], F32, tag="bnst")
                for r in range(R):
                    nc.vector.bn_stats(out=st[:, r, 0, :], in_=xt[:, r, 0:512])
                    nc.vector.bn_stats(out=st[:, r, 1, :], in_=xt[:, r, 512:1024])
                mv = stats.tile([P, R, 2], F32, tag="mv")
                for r in range(R):
                    nc.vector.bn_aggr(out=mv[:, r, :], in_=st[:, r, :, :])

                # std = sqrt(var * n/(n-1)) for all rows in one ACT op
                std = stats.tile([P, R], F32, tag="std")
                nc.scalar.activation(
                    out=std,
                    in_=mv[:, :, 1],
                    func=mybir.ActivationFunctionType.Sqrt,
                    scale=DDOF_SCALE,
                )
                rstd = stats.tile([P, R], F32, tag="rstd")
                nc.vector.reciprocal(out=rstd, in_=std)
                # negm = -mean (ACT); nmr[r] = -m*rstd via ACT Copy(scale=rstd)
                negm = stats.tile([P, R], F32, tag="negm")
                nc.scalar.activation(
                    out=negm,
                    in_=mv[:, :, 0],
                    func=mybir.ActivationFunctionType.Copy,
                    scale=-1.0,
                )
                nmr = stats.tile([P, R], F32, tag="nmr")
                for r in range(R):
                    nc.scalar.activation(
                        out=nmr[:, r : r + 1],
                        in_=negm[:, r : r + 1],
                        func=mybir.ActivationFunctionType.Copy,
                        scale=rstd[:, r : r + 1],
                    )

                # t = x * rstd + (-m * rstd)  (normalize on ACT; frees xt early)
                t = tmp.tile([P, R, D], F32, tag="t")
                for r in range(R):
                    nc.scalar.activation(
                        out=t[:, r, :],
                        in_=xt[:, r, :],
                        func=mybir.ActivationFunctionType.Identity,
                        bias=nmr[:, r : r + 1],
                        scale=rstd[:, r : r + 1],
                    )
                # t *= w  (DVE, all rows, in place)
                nc.vector.tensor_mul(out=t, in0=t, in1=Wb)
                # t += b  (GPSIMD, all rows, in place)
                nc.gpsimd.tensor_add(out=t, in0=t, in1=Bb)
                nc.sync.dma_start(out=o_t[i], in_=t)

    nc.compile()
    return nc


def _get_nc() -> bass.Bass:
    global _nc_cache
    if _nc_cache is None:
        _nc_cache = _build_nc()
    return _nc_cache


def run(X, weights, bias, **spmd_kwargs):
    """Run on 8 cores; returns (full_output, BassKernelResults)."""
    X = np.ascontiguousarray(X, dtype=np.float32)
    w = np.ascontiguousarray(weights, dtype=np.float32)
    b = np.ascontiguousarray(bias, dtype=np.float32)
    assert X.shape == (BATCH, D) and w.shape == (D,) and b.shape == (D,)

    nc = _get_nc()
    shards = np.split(X, N_CORES, axis=0)
    in_maps = [{"X": shards[c], "W": w, "B": b} for c in range(N_CORES)]
    res = run_bass_kernel_spmd(nc, in_maps, core_ids=list(range(N_CORES)), **spmd_kwargs)
    out = np.concatenate([res.results[c]["out"] for c in range(N_CORES)], axis=0)
    return out, res


def kernel(X, weights, bias):
    out, _ = run(X, weights, bias)
    return out


# revision 17
# speedup vs baseline: 1.2242x; 1.0459x over previous
"""ContextNorm (row-wise layernorm w/ ddof=1 + diag scale + bias) on 8 TRN2 cores.

out = (X - mean(X, axis=1)) / std(X, axis=1, ddof=1) * weights + bias

Sharding: data-parallel over the batch axis (65536 rows -> 8192 rows/core);
weights/bias replicated. Per core: 16 tiles of [128 partitions x 4 rows x 1024]
(16KB contiguous per partition per DMA). Stats via bn_stats/bn_aggr (DVE),
normalize on ACT (in place), *W on DVE (in place), +B on GPSIMD; loads ride
the SP HWDGE ring, stores the ACT HWDGE ring.
"""

import sys

sys.path.insert(0, "/opt/trn_rl_repo")

import numpy as np

import concourse.bass as bass
import concourse.bacc as bacc
import concourse.tile as tile
from concourse import mybir
from concourse.bass_utils import run_bass_kernel_spmd

N_CORES = 8
BATCH = 65536
D = 1024
ROWS = BATCH // N_CORES  # 8192 rows per core
P = 128
R = 4  # rows per partition per tile
N_TILES = ROWS // (P * R)  # 16
F32 = mybir.dt.float32
DDOF_SCALE = float(D) / float(D - 1)  # biased var -> unbiased (ddof=1)

_nc_cache = None


def _broadcast_ap(ap: bass.AP, p: int, r: int) -> bass.AP:
    # Replicate a 1-D [D] DRAM tensor to [p, r, D] (step-0 leading dims).
    return bass.AP(tensor=ap.tensor, offset=ap.offset, ap=[[0, p], [0, r], *ap.ap])


def _build_nc() -> bass.Bass:
    nc = bacc.Bacc("TRN2", target_bir_lowering=False)
    X = nc.dram_tensor("X", [ROWS, D], F32, kind="ExternalInput")
    W = nc.dram_tensor("W", [D], F32, kind="ExternalInput")
    B = nc.dram_tensor("B", [D], F32, kind="ExternalInput")
    O = nc.dram_tensor("out", [ROWS, D], F32, kind="ExternalOutput")

    x_t = X[:, :].rearrange("(n p r) d -> n p r d", p=P, r=R)
    o_t = O[:, :].rearrange("(n p r) d -> n p r d", p=P, r=R)

    with tile.TileContext(nc) as tc:
        with (
            tc.tile_pool(name="consts", bufs=1) as consts,
            tc.tile_pool(name="xio", bufs=4) as xio,
            tc.tile_pool(name="tmp", bufs=5) as tmp,
            tc.tile_pool(name="stats", bufs=16) as stats,
        ):
            Wb = consts.tile([P, R, D], F32)
            Bb = consts.tile([P, R, D], F32)
            nc.gpsimd.dma_start(out=Wb, in_=_broadcast_ap(W[:], P, R))
            nc.gpsimd.dma_start(out=Bb, in_=_broadcast_ap(B[:], P, R))

            for i in range(N_TILES):
                xt = xio.tile([P, R, D], F32, tag="x")
                nc.sync.dma_start(out=xt, in_=x_t[i])

                st = stats.tile([P, R, 2, 6], F32, tag="bnst")
                for r in range(R):
                    nc.vector.bn_stats(out=st[:, r, 0, :], in_=xt[:, r, 0:512])
                    nc.vector.bn_stats(out=st[:, r, 1, :], in_=xt[:, r, 512:1024])
                mv = stats.tile([P, R, 2], F32, tag="mv")
                for r in range(R):
                    nc.vector.bn_aggr(out=mv[:, r, :], in_=st[:, r, :, :])

                # std = sqrt(var * n/(n-1)) for all rows in one ACT op
                std = stats.tile([P, R], F32, tag="std")
                nc.scalar.activation(
                    out=std,
                    in_=mv[:, :, 1],
                    func=mybir.ActivationFunctionType.Sqrt,
                    scale=DDOF_SCALE,
                )
                rstd = stats.tile([P, R], F32, tag="rstd")
                nc.vector.reciprocal(out=rstd, in_=std)

                # t = (x - m) * w   (DVE fused; frees xt early)
                t = tmp.tile([P, R, D], F32, tag="t")
                for r in range(R):
                    nc.vector.scalar_tensor_tensor(
                        out=t[:, r, :],
                        in0=xt[:, r, :],
                        scalar=mv[:, r, 0:1],
                        in1=Wb[:, r, :],
                        op0=mybir.AluOpType.subtract,
                        op1=mybir.AluOpType.mult,
                    )
                # t = t * rstd + b  (DVE fused, in place)
                for r in range(R):
                    nc.vector.scalar_tensor_tensor(
                        out=t[:, r, :],
                        in0=t[:, r, :],
                        scalar=rstd[:, r : r + 1],
                        in1=Bb[:, r, :],
                        op0=mybir.AluOpType.mult,
                        op1=mybir.AluOpType.add,
                    )
                nc.sync.dma_start(out=o_t[i], in_=t)

    nc.compile()
    return nc


def _get_nc() -> bass.Bass:
    global _nc_cache
    if _nc_cache is None:
        _nc_cache = _build_nc()
    return _nc_cache


def run(X, weights, bias, **spmd_kwargs):
    """Run on 8 cores; returns (full_output, BassKernelResults)."""
    X = np.ascontiguousarray(X, dtype=np.float32)
    w = np.ascontiguousarray(weights, dtype=np.float32)
    b = np.ascontiguousarray(bias, dtype=np.float32)
    assert X.shape == (BATCH, D) and w.shape == (D,) and b.shape == (D,)

    nc = _get_nc()
    shards = np.split(X, N_CORES, axis=0)
    in_maps = [{"X": shards[c], "W": w, "B": b} for c in range(N_CORES)]
    res = run_bass_kernel_spmd(nc, in_maps, core_ids=list(range(N_CORES)), **spmd_kwargs)
    out = np.concatenate([res.results[c]["out"] for c in range(N_CORES)], axis=0)
    return out, res


def kernel(X, weights, bias):
    out, _ = run(X, weights, bias)
    return out


# revision 19
# speedup vs baseline: 1.4045x; 1.1472x over previous
"""ContextNorm (row-wise layernorm w/ ddof=1 + diag scale + bias) on 8 TRN2 cores.

out = (X - mean(X, axis=1)) / std(X, axis=1, ddof=1) * weights + bias

Sharding: data-parallel over the batch axis (65536 rows -> 8192 rows/core);
weights/bias replicated. Per core: 16 tiles of [128 partitions x 4 rows x 1024]
(16KB contiguous per partition per DMA). Stats via bn_stats/bn_aggr (DVE),
normalize on ACT (in place), *W on DVE (in place), +B on GPSIMD; loads ride
the SP HWDGE ring, stores the ACT HWDGE ring.
"""

import sys

sys.path.insert(0, "/opt/trn_rl_repo")

import numpy as np

import concourse.bass as bass
import concourse.bacc as bacc
import concourse.tile as tile
from concourse import mybir
from concourse.bass_utils import run_bass_kernel_spmd

N_CORES = 8
BATCH = 65536
D = 1024
ROWS = BATCH // N_CORES  # 8192 rows per core
P = 128
R = 4  # rows per partition per tile
N_TILES = ROWS // (P * R)  # 16
F32 = mybir.dt.float32
DDOF_SCALE = float(D) / float(D - 1)  # biased var -> unbiased (ddof=1)

_nc_cache = None


def _broadcast_ap(ap: bass.AP, p: int, r: int) -> bass.AP:
    # Replicate a 1-D [D] DRAM tensor to [p, r, D] (step-0 leading dims).
    return bass.AP(tensor=ap.tensor, offset=ap.offset, ap=[[0, p], [0, r], *ap.ap])


def _build_nc() -> bass.Bass:
    nc = bacc.Bacc("TRN2", target_bir_lowering=False)
    X = nc.dram_tensor("X", [ROWS, D], F32, kind="ExternalInput")
    W = nc.dram_tensor("W", [D], F32, kind="ExternalInput")
    B = nc.dram_tensor("B", [D], F32, kind="ExternalInput")
    O = nc.dram_tensor("out", [ROWS, D], F32, kind="ExternalOutput")

    x_t = X[:, :].rearrange("(n p r) d -> n p r d", p=P, r=R)
    o_t = O[:, :].rearrange("(n p r) d -> n p r d", p=P, r=R)

    with tile.TileContext(nc) as tc:
        with (
            tc.tile_pool(name="consts", bufs=1) as consts,
            tc.tile_pool(name="xio", bufs=4) as xio,
            tc.tile_pool(name="tmp", bufs=5) as tmp,
            tc.tile_pool(name="scratch", bufs=2) as scratch,
            tc.tile_pool(name="stats", bufs=16) as stats,
        ):
            Wb = consts.tile([P, R, D], F32)
            Bb = consts.tile([P, R, D], F32)
            nc.gpsimd.dma_start(out=Wb, in_=_broadcast_ap(W[:], P, R))
            nc.gpsimd.dma_start(out=Bb, in_=_broadcast_ap(B[:], P, R))

            for i in range(N_TILES):
                xt = xio.tile([P, R, D], F32, tag="x")
                nc.sync.dma_start(out=xt, in_=x_t[i])

                # Rows 0-1: bn_stats/bn_aggr on DVE. Rows 2-3: sum/sumsq via
                # ACT accumulate passes (offloads DVE, the critical engine).
                st = stats.tile([P, 2, 2, 6], F32, tag="bnst")
                for r in range(2):
                    nc.vector.bn_stats(out=st[:, r, 0, :], in_=xt[:, r, 0:512])
                    nc.vector.bn_stats(out=st[:, r, 1, :], in_=xt[:, r, 512:1024])
                mv = stats.tile([P, 2, 2], F32, tag="mv")
                for r in range(2):
                    nc.vector.bn_aggr(out=mv[:, r, :], in_=st[:, r, :, :])

                sums = stats.tile([P, 2], F32, tag="sums")
                sqs = stats.tile([P, 2], F32, tag="sqs")
                scr = scratch.tile([P, D], F32, tag="scr")
                scr2 = scratch.tile([P, D], F32, tag="scr2")
                for j, r in enumerate((2, 3)):
                    nc.scalar.activation(
                        out=scr,
                        in_=xt[:, r, :],
                        func=mybir.ActivationFunctionType.Copy,
                        accum_out=sums[:, j : j + 1],
                    )
                    nc.scalar.activation(
                        out=scr2,
                        in_=xt[:, r, :],
                        func=mybir.ActivationFunctionType.Square,
                        accum_out=sqs[:, j : j + 1],
                    )
                # m23 = sums/D ;  a = sums^2/D ;  u = sqs - a ; std23 = sqrt(u/(D-1))
                m23 = stats.tile([P, 2], F32, tag="m23")
                nc.scalar.activation(
                    out=m23,
                    in_=sums,
                    func=mybir.ActivationFunctionType.Copy,
                    scale=1.0 / D,
                )
                a23 = stats.tile([P, 2], F32, tag="a23")
                nc.scalar.activation(
                    out=a23,
                    in_=sums,
                    func=mybir.ActivationFunctionType.Square,
                    scale=1.0 / np.sqrt(D),
                )
                u23 = stats.tile([P, 2], F32, tag="u23")
                nc.gpsimd.tensor_tensor(
                    out=u23, in0=sqs, in1=a23, op=mybir.AluOpType.subtract
                )

                # std for all rows: rows 0-1 from biased var, rows 2-3 from u
                std = stats.tile([P, R], F32, tag="std")
                nc.scalar.activation(
                    out=std[:, 0:2],
                    in_=mv[:, :, 1],
                    func=mybir.ActivationFunctionType.Sqrt,
                    scale=DDOF_SCALE,
                )
                nc.scalar.activation(
                    out=std[:, 2:4],
                    in_=u23,
                    func=mybir.ActivationFunctionType.Sqrt,
                    scale=1.0 / (D - 1),
                )
                rstd = stats.tile([P, R], F32, tag="rstd")
                nc.vector.reciprocal(out=rstd, in_=std)

                # t = (x - m) * w   (DVE fused; frees xt early)
                t = tmp.tile([P, R, D], F32, tag="t")
                for r in range(R):
                    mean_ap = mv[:, r, 0:1] if r < 2 else m23[:, r - 2 : r - 1]
                    nc.vector.scalar_tensor_tensor(
                        out=t[:, r, :],
                        in0=xt[:, r, :],
                        scalar=mean_ap,
                        in1=Wb[:, r, :],
                        op0=mybir.AluOpType.subtract,
                        op1=mybir.AluOpType.mult,
                    )
                # t = t * rstd + b  (DVE fused, in place)
                for r in range(R):
                    nc.vector.scalar_tensor_tensor(
                        out=t[:, r, :],
                        in0=t[:, r, :],
                        scalar=rstd[:, r : r + 1],
                        in1=Bb[:, r, :],
                        op0=mybir.AluOpType.mult,
                        op1=mybir.AluOpType.add,
                    )
                nc.sync.dma_start(out=o_t[i], in_=t)

    nc.compile()
    return nc


def _get_nc() -> bass.Bass:
    global _nc_cache
    if _nc_cache is None:
        _nc_cache = _build_nc()
    return _nc_cache


def run(X, weights, bias, **spmd_kwargs):
    """Run on 8 cores; returns (full_output, BassKernelResults)."""
    X = np.ascontiguousarray(X, dtype=np.float32)
    w = np.ascontiguousarray(weights, dtype=np.float32)
    b = np.ascontiguousarray(bias, dtype=np.float32)
    assert X.shape == (BATCH, D) and w.shape == (D,) and b.shape == (D,)

    nc = _get_nc()
    shards = np.split(X, N_CORES, axis=0)
    in_maps = [{"X": shards[c], "W": w, "B": b} for c in range(N_CORES)]
    res = run_bass_kernel_spmd(nc, in_maps, core_ids=list(range(N_CORES)), **spmd_kwargs)
    out = np.concatenate([res.results[c]["out"] for c in range(N_CORES)], axis=0)
    return out, res


def kernel(X, weights, bias):
    out, _ = run(X, weights, bias)
    return out
